# revision 21
# baseline (speedup 1.0000x reference)
"""Trainium2 Bass kernel for nn_AdaptiveSparseUpdateRule.

Reference, per pixel of a [B=16, C=16, H=256, W=256] grid:
  feats = [x, sobel_x(x), sobel_y(x)]            (depthwise 3x3, SAME)
  h = relu(feats @ w1 + b1); h = relu(h @ w2 + b2); u = h @ w3 + b3
  alive = maxpool3x3(x[:,3]) > 0.1
  out = u * (fire_mask * alive != 0)

Layout/strategy (8 cores, data-parallel over batch, 2 images/core):
- Channel-major: channels on partitions, pixels on the free axis; the MLP
  runs on 512-pixel groups (2 image rows).
- Sobel convs folded into matmul 1: K=48 operand = 3 row-shifted copies of
  x (dy blocks on partitions); the 3 column shifts are 3 PSUM-accumulated
  matmuls reading the same tile at free offsets, with host-precomputed
  weights W[dy,dx] = 1[dy=dx=0] w1_a + kx[dy,dx] w1_b + ky[dy,dx] w1_c.
- x is pre-padded on the host to [C, H+2, W+2] bf16 (zero border), so
  every tile load is one contiguous-per-partition DMA with no edge cases,
  and SAME-padding falls out of the layout.
- Groups processed in pairs: group g on partitions 0-47, g+1 on 64-111;
  matmul-1s use tile_position (0,0)/(64,0) and overlap on disjoint PE row
  strips (~117ns/mm measured). Matmul-3 (M=16) is col-packed 4 groups to
  a PSUM bank via tile_position (0,32j) (~55ns/mm measured).
- bf16 matmuls (fp32r loses FWL and runs no faster), fp32 PSUM/epilogue.
- alive/fire mask computed once per image in a strip layout (partition =
  2-row group), staged to DRAM as bf16, broadcast per 4-group block with
  one stride-0 DMA.
- relu1 on ACT, relu2 on DVE, mask pipeline on gpsimd, DMA triggers
  spread over sync/gpsimd (each trigger costs ~0.6us on its engine).
"""

import numpy as np
import ml_dtypes

import concourse.bass as bass
import concourse.mybir as mybir
import concourse.tile as tile
from concourse import bacc
from concourse.bass_utils import run_bass_kernel_spmd

F32 = mybir.dt.float32
BF16 = mybir.dt.bfloat16
AF = mybir.ActivationFunctionType
ALU = mybir.AluOpType

N_CORES = 8
B, C, H, W = 16, 16, 256, 256
EMB = 128
B_LOC = B // N_CORES
WP = W + 2


def _mkap(base, extra_offset, dims):
    """Raw access pattern on base's tensor: dims = [[step, num], ...] (elems)."""
    return bass.AP(tensor=base.tensor, offset=base.offset + extra_offset, ap=dims)


def build(b_loc=B_LOC, h=H, relu2_act_every=4):
    nc = bacc.Bacc("TRN2", target_bir_lowering=False, debug=False, num_devices=N_CORES)

    xp_d = nc.declare_dram_parameter("xp", [b_loc, C, h + 2, WP], BF16, isOutput=False)
    alpha_d = nc.declare_dram_parameter("alpha", [b_loc, h, W], F32, isOutput=False)
    fire_d = nc.declare_dram_parameter("fire", [b_loc, h, W], F32, isOutput=False)
    wcat_d = nc.declare_dram_parameter("wcat", [128, 3, EMB], BF16, isOutput=False)
    w2_d = nc.declare_dram_parameter("w2", [EMB, EMB], BF16, isOutput=False)
    w3_d = nc.declare_dram_parameter("w3", [EMB, 32], BF16, isOutput=False)
    b1_d = nc.declare_dram_parameter("b1", [EMB, 1], F32, isOutput=False)
    b2_d = nc.declare_dram_parameter("b2", [EMB, 1], F32, isOutput=False)
    b3r_d = nc.declare_dram_parameter("b3r", [128, 1], F32, isOutput=False)
    out_d = nc.declare_dram_parameter("out", [b_loc, h // 8, 128, 512], F32, isOutput=True)

    nst = h // 2  # 2-row groups per image
    assert nst % 4 == 0

    with tile.TileContext(nc) as tc:
        with (
            tc.tile_pool(name="const", bufs=1) as const,
            tc.tile_pool(name="mask", bufs=2) as mask,
            tc.tile_pool(name="dram", bufs=2, space="DRAM") as dramp,
            tc.tile_pool(name="xtp", bufs=4) as xtp,
            tc.tile_pool(name="act", bufs=4) as act,
            tc.tile_pool(name="outp", bufs=3) as outp,
            tc.tile_pool(name="ps1", bufs=2, space="PSUM") as ps1p,
            tc.tile_pool(name="ps2", bufs=1, space="PSUM") as ps2p,
            tc.tile_pool(name="ps3", bufs=2, space="PSUM") as ps3p,
        ):
            wcat_t = const.tile([128, 3, EMB], BF16)
            w2_t = const.tile([EMB, EMB], BF16)
            w3_t = const.tile([EMB, 32], BF16)
            b1_t = const.tile([EMB, 1], F32)
            b2_t = const.tile([EMB, 1], F32)
            b3r_t = const.tile([128, 1], F32)
            nc.sync.dma_start(out=wcat_t[:], in_=wcat_d[:])
            nc.sync.dma_start(out=w2_t[:], in_=w2_d[:])
            nc.sync.dma_start(out=w3_t[:], in_=w3_d[:])
            nc.sync.dma_start(out=b1_t[:], in_=b1_d[:])
            nc.sync.dma_start(out=b2_t[:], in_=b2_d[:])
            nc.sync.dma_start(out=b3r_t[:], in_=b3r_d[:])

            def emit_mask(b):
                # sel[g] = (maxpool3x3(alpha) > 0.1) * fire, staged to DRAM bf16
                alpha = alpha_d[b]
                am = mask.tile([128, 4, WP], F32, tag="am")
                nc.gpsimd.memset(am[:], 0.0)
                # center rows contiguously, halo rows via partition shifts
                nc.sync.dma_start(
                    out=am[0:nst, 1:3, 1 : W + 1],
                    in_=alpha.rearrange("(p t) w -> p t w", t=2),
                )
                nc.sync.dma_start(out=am[1:nst, 0, :], in_=am[0 : nst - 1, 2, :])
                nc.sync.dma_start(out=am[0 : nst - 1, 3, :], in_=am[1:nst, 1, :])
                mh = mask.tile([128, 4, W], F32, tag="mh")
                nc.vector.tensor_tensor(mh[0:nst], am[0:nst, :, 0:W], am[0:nst, :, 1 : W + 1], ALU.max)
                nc.vector.tensor_tensor(mh[0:nst], mh[0:nst], am[0:nst, :, 2 : W + 2], ALU.max)
                mv = mask.tile([128, 2, W], F32, tag="mv")
                nc.vector.tensor_tensor(mv[0:nst], mh[0:nst, 0:2, :], mh[0:nst, 1:3, :], ALU.max)
                nc.vector.tensor_tensor(mv[0:nst], mv[0:nst], mh[0:nst, 2:4, :], ALU.max)
                fire_t = mask.tile([128, 2, W], F32, tag="fire")
                nc.sync.dma_start(out=fire_t[0:nst], in_=fire_d[b].rearrange("(p t) w -> p t w", t=2))
                sel_t = mask.tile([128, 2, W], BF16, tag="sel")
                nc.vector.scalar_tensor_tensor(
                    out=sel_t[0:nst], in0=mv[0:nst], scalar=0.1, in1=fire_t[0:nst],
                    op0=ALU.is_gt, op1=ALU.mult,
                )
                sel_dd = dramp.tile([nst, 2 * W], BF16, tag="seld", name=f"seld{b}")
                nc.sync.dma_start(out=sel_dd[:], in_=sel_t[0:nst].rearrange("p t w -> p (t w)"))
                return sel_dd

            # ---- software-pipelined MLP emission ----
            # pair = 2 groups, 4 image rows apart (rows rr..rr+1 and
            # rr+4..rr+5) loaded by ONE 7-block sliding-window DMA (junk
            # block fills the partition gap 48-63).  Stages are emitted one
            # pair apart so the in-order PE queue never waits on a fresh
            # ACT/DVE product.
            npair_img = nst // 2
            NP = b_loc * npair_img
            st = {}
            sel_dds = {}
            ps3s = {}
            xts = {}

            # PE warmup operand: no-dep matmuls keep the HAM clock-gate
            # open while the pipeline fills (emitted inside early stage1s,
            # overwritten by the real start=True matmuls)
            zt = const.tile([128, 64], BF16)
            nc.vector.memset(zt[:], 0.0)

            def stage1(idx):
                b, pl = divmod(idx, npair_img)
                sub = pl % 2

                def fetch_xt(bb, blkno):
                    xpb = xp_d[bb]
                    ch_stride = (h + 2) * WP
                    xt = xtp.tile([128, 4, WP], BF16, tag="xt", name="xt")
                    src = _mkap(
                        xpb, 8 * blkno * WP,
                        [[WP, 7], [ch_stride, C], [WP, 4], [1, WP]],
                    )
                    eng = nc.sync if blkno % 2 == 0 else nc.gpsimd
                    eng.dma_start(out=xt[0:112, :, :], in_=src)
                    xts[(bb, blkno)] = xt

                if (b, pl // 2) not in xts:
                    fetch_xt(b, pl // 2)
                xt = xts[(b, pl // 2)] if sub == 0 else xts.pop((b, pl // 2))
                if sub == 1 and idx + 2 < NP:
                    # prefetch the tile needed ~2 pairs ahead
                    nb, npl = divmod(idx + 2, npair_img)
                    nblk = npl // 2
                    if (nb, nblk) not in xts:
                        fetch_xt(nb, nblk)
                ps1 = ps1p.tile([128, 2, 512], F32)
                if idx < 8:
                    for _ in range(12):
                        nc.tensor.matmul(
                            out=ps1[0:64, 0, 0:64], lhsT=zt[:], rhs=zt[:],
                            start=True, stop=True,
                        )
                for i in range(3):
                    for half in range(2):
                        pbase = 64 * half
                        nc.tensor.matmul(
                            out=ps1[:, half, :],
                            lhsT=wcat_t[pbase : pbase + 48, i, :],
                            rhs=xt[pbase : pbase + 48, 2 * sub : 2 * sub + 2, i : W + i],
                            start=(i == 0), stop=(i == 2),
                            tile_position=(pbase, 0),
                        )
                h1 = act.tile([EMB, 2, 512], BF16, tag="h1", name="h1")
                nc.scalar.activation(out=h1[:], in_=ps1[:], func=AF.Relu, bias=b1_t[:])
                st[idx] = [h1]

            def stage2(idx):
                (h1,) = st[idx]
                ps2 = ps2p.tile([128, 2, 512], F32)
                for half in range(2):
                    nc.tensor.matmul(
                        out=ps2[:, half, :], lhsT=w2_t[:], rhs=h1[:, half, :],
                        start=True, stop=True,
                    )
                h2 = act.tile([EMB, 2, 512], BF16, tag="h2", name="h2")
                if relu2_act_every and idx % relu2_act_every == 0:
                    nc.scalar.activation(out=h2[:], in_=ps2[:], func=AF.Relu, bias=b2_t[:])
                else:
                    nc.vector.tensor_scalar(h2[:], ps2[:], b2_t[:], 0.0, ALU.add, ALU.max)
                st[idx] = [h2]

            def stage3(idx):
                b, pl = divmod(idx, npair_img)
                (h2,) = st.pop(idx)
                blk = pl // 2
                sub = pl % 2
                if sub == 0:
                    ps3s[(b, blk)] = ps3p.tile([128, 512], F32, name="ps3")
                ps3 = ps3s[(b, blk)]
                for half in range(2):
                    j = 2 * half + sub
                    nc.tensor.matmul(
                        out=ps3[32 * j : 32 * j + 32, :], lhsT=w3_t[:],
                        rhs=h2[:, half, :],
                        start=True, stop=True, tile_position=(0, 32 * j),
                    )
                if sub == 1:
                    ps3s.pop((b, blk))
                    g0 = 4 * blk
                    selb = outp.tile([128, 512], BF16, tag="selb", name="selb")
                    nc.gpsimd.dma_start(
                        out=selb[:],
                        in_=sel_dds[b][g0 : g0 + 4, None, :].to_broadcast([4, 32, 2 * W]),
                    )
                    osb = outp.tile([128, 512], F32, tag="osb", name="osb")
                    nc.vector.scalar_tensor_tensor(
                        out=osb[:], in0=ps3[:], scalar=b3r_t[:], in1=selb[:],
                        op0=ALU.add, op1=ALU.mult,
                    )
                    eng = nc.sync if blk % 2 == 0 else nc.gpsimd
                    eng.dma_start(out=out_d[b, blk], in_=osb[:])

            sel_dds[0] = emit_mask(0)
            for p in range(NP + 2):
                if p < NP:
                    stage1(p)
                if p == npair_img // 2 and b_loc > 1:
                    sel_dds[1] = emit_mask(1)
                if 1 <= p <= NP:
                    stage2(p - 1)
                if 2 <= p <= NP + 1:
                    stage3(p - 2)

    nc.compile()
    return nc


def host_weights(w1, b1, w2, b2, w3, b3):
    sob = np.array([[-1.0, 0, 1], [-2, 0, 2], [-1, 0, 1]], np.float32)
    kx, ky = sob, sob.T
    w1 = np.asarray(w1, np.float32)
    w1a, w1b, w1c = w1[0:C], w1[C : 2 * C], w1[2 * C : 3 * C]
    wcat48 = np.zeros((48, 3, EMB), np.float32)
    for i, dx in enumerate((-1, 0, 1)):
        for blk, dy in enumerate((-1, 0, 1)):
            m = kx[dy + 1, dx + 1] * w1b + ky[dy + 1, dx + 1] * w1c
            if dy == 0 and dx == 0:
                m = m + w1a
            wcat48[16 * blk : 16 * blk + 16, i, :] = m
    wcat = np.zeros((128, 3, EMB), np.float32)
    wcat[0:48] = wcat48
    wcat[64:112] = wcat48
    b3r = np.zeros((128, 1), np.float32)
    for j in range(4):
        b3r[32 * j : 32 * j + 16, 0] = np.asarray(b3, np.float32).reshape(C)
    return {
        "wcat": wcat.astype(ml_dtypes.bfloat16),
        "w2": np.asarray(w2, np.float32).astype(ml_dtypes.bfloat16),
        "w3": np.pad(np.asarray(w3, np.float32), ((0, 0), (0, 16))).astype(
            ml_dtypes.bfloat16
        ),
        "b1": np.asarray(b1, np.float32).reshape(EMB, 1),
        "b2": np.asarray(b2, np.float32).reshape(EMB, 1),
        "b3r": b3r,
    }


def host_x(x, h=H):
    """Pad to [*, C, h+2, W+2] bf16 with a zero border."""
    b = x.shape[0]
    xp = np.zeros((b, C, h + 2, WP), ml_dtypes.bfloat16)
    xp[:, :, 1 : h + 1, 1 : W + 1] = x.astype(ml_dtypes.bfloat16)
    return xp


_nc_cache = {}


def _get_nc():
    if "nc" not in _nc_cache:
        _nc_cache["nc"] = build()
    return _nc_cache["nc"]


def make_in_maps(x, fire_mask, w1, b1, w2, b2, w3, b3):
    x = np.ascontiguousarray(np.asarray(x), np.float32)
    fire = np.ascontiguousarray(np.asarray(fire_mask), np.float32)
    wts = host_weights(w1, b1, w2, b2, w3, b3)
    xp = host_x(x)
    in_maps = []
    for c in range(N_CORES):
        sl = slice(B_LOC * c, B_LOC * (c + 1))
        in_maps.append({
            "xp": xp[sl],
            "alpha": np.ascontiguousarray(x[sl, 3]),
            "fire": np.ascontiguousarray(fire[sl, 0]),
            **wts,
        })
    return in_maps


def unstage(out_stage, h=H):
    """[b, h//8, 128, 512] staging -> [b, C, h, W]."""
    b = out_stage.shape[0]
    v = out_stage.reshape(b, h // 8, 4, 32, 2, W)[:, :, :, 0:C]
    return np.ascontiguousarray(
        v.transpose(0, 3, 1, 2, 4, 5).reshape(b, C, h, W)
    )


def kernel(x, fire_mask, w1, b1, w2, b2, w3, b3):
    nc = _get_nc()
    in_maps = make_in_maps(x, fire_mask, w1, b1, w2, b2, w3, b3)
    res = run_bass_kernel_spmd(nc, in_maps, core_ids=list(range(N_CORES)))
    return np.concatenate(
        [unstage(res.results[c]["out"]) for c in range(N_CORES)], axis=0
    )


# revision 23
# speedup vs baseline: 1.1646x; 1.1646x over previous
"""Trainium2 Bass kernel for nn_AdaptiveSparseUpdateRule.

Reference, per pixel of a [B=16, C=16, H=256, W=256] grid:
  feats = [x, sobel_x(x), sobel_y(x)]            (depthwise 3x3, SAME)
  h = relu(feats @ w1 + b1); h = relu(h @ w2 + b2); u = h @ w3 + b3
  alive = maxpool3x3(x[:,3]) > 0.1
  out = u * (fire_mask * alive != 0)

Layout/strategy (8 cores, data-parallel over batch, 2 images/core):
- Channel-major: channels on partitions, pixels on the free axis; the MLP
  runs on 512-pixel groups (2 image rows).
- Sobel convs folded into matmul 1: K=48 operand = 3 row-shifted copies of
  x (dy blocks on partitions); the 3 column shifts are 3 PSUM-accumulated
  matmuls reading the same tile at free offsets, with host-precomputed
  weights W[dy,dx] = 1[dy=dx=0] w1_a + kx[dy,dx] w1_b + ky[dy,dx] w1_c.
- x is pre-padded on the host to [C, H+2, W+2] bf16 (zero border), so
  every tile load is one contiguous-per-partition DMA with no edge cases,
  and SAME-padding falls out of the layout.
- Groups processed in pairs: group g on partitions 0-47, g+1 on 64-111;
  matmul-1s use tile_position (0,0)/(64,0) and overlap on disjoint PE row
  strips (~117ns/mm measured). Matmul-3 (M=16) is col-packed 4 groups to
  a PSUM bank via tile_position (0,32j) (~55ns/mm measured).
- bf16 matmuls (fp32r loses FWL and runs no faster), fp32 PSUM/epilogue.
- alive/fire mask computed once per image in a strip layout (partition =
  2-row group), staged to DRAM as bf16, broadcast per 4-group block with
  one stride-0 DMA.
- relu1 on ACT, relu2 on DVE, mask pipeline on gpsimd, DMA triggers
  spread over sync/gpsimd (each trigger costs ~0.6us on its engine).
"""

import numpy as np
import ml_dtypes

import concourse.bass as bass
import concourse.mybir as mybir
import concourse.tile as tile
from concourse import bacc
from concourse.bass_utils import run_bass_kernel_spmd

F32 = mybir.dt.float32
BF16 = mybir.dt.bfloat16
AF = mybir.ActivationFunctionType
ALU = mybir.AluOpType

N_CORES = 8
B, C, H, W = 16, 16, 256, 256
EMB = 128
B_LOC = B // N_CORES
WP = W + 2


def _mkap(base, extra_offset, dims):
    """Raw access pattern on base's tensor: dims = [[step, num], ...] (elems)."""
    return bass.AP(tensor=base.tensor, offset=base.offset + extra_offset, ap=dims)


def build(b_loc=B_LOC, h=H, relu2_act_every=4):
    nc = bacc.Bacc("TRN2", target_bir_lowering=False, debug=False, num_devices=N_CORES)

    xp_d = nc.declare_dram_parameter("xp", [b_loc, C, h + 2, WP], BF16, isOutput=False)
    alpha_d = nc.declare_dram_parameter("alpha", [b_loc, h, W], F32, isOutput=False)
    fire_d = nc.declare_dram_parameter("fire", [b_loc, h, W], F32, isOutput=False)
    wcat_d = nc.declare_dram_parameter("wcat", [128, 3, EMB], BF16, isOutput=False)
    w2_d = nc.declare_dram_parameter("w2", [EMB, EMB], BF16, isOutput=False)
    w3_d = nc.declare_dram_parameter("w3", [EMB, 32], BF16, isOutput=False)
    b1_d = nc.declare_dram_parameter("b1", [EMB, 1], F32, isOutput=False)
    b2_d = nc.declare_dram_parameter("b2", [EMB, 1], F32, isOutput=False)
    b3r_d = nc.declare_dram_parameter("b3r", [128, 1], F32, isOutput=False)
    out_d = nc.declare_dram_parameter("out", [b_loc, h // 8, 128, 512], F32, isOutput=True)

    nst = h // 2  # 2-row groups per image
    assert nst % 4 == 0

    with tile.TileContext(nc) as tc:
        with (
            tc.tile_pool(name="const", bufs=1) as const,
            tc.tile_pool(name="mask", bufs=2) as mask,
            tc.tile_pool(name="dram", bufs=2, space="DRAM") as dramp,
            tc.tile_pool(name="xtp", bufs=4) as xtp,
            tc.tile_pool(name="act", bufs=4) as act,
            tc.tile_pool(name="outp", bufs=3) as outp,
            tc.tile_pool(name="ps1", bufs=2, space="PSUM") as ps1p,
            tc.tile_pool(name="ps2", bufs=1, space="PSUM") as ps2p,
            tc.tile_pool(name="ps3", bufs=2, space="PSUM") as ps3p,
        ):
            wcat_t = const.tile([128, 3, EMB], BF16)
            w2_t = const.tile([EMB, EMB], BF16)
            w3_t = const.tile([EMB, 32], BF16)
            b1_t = const.tile([EMB, 1], F32)
            b2_t = const.tile([EMB, 1], F32)
            b3r_t = const.tile([128, 1], F32)
            nc.sync.dma_start(out=wcat_t[:], in_=wcat_d[:])
            nc.sync.dma_start(out=w2_t[:], in_=w2_d[:])
            nc.sync.dma_start(out=w3_t[:], in_=w3_d[:])
            nc.sync.dma_start(out=b1_t[:], in_=b1_d[:])
            nc.sync.dma_start(out=b2_t[:], in_=b2_d[:])
            nc.sync.dma_start(out=b3r_t[:], in_=b3r_d[:])

            def emit_mask(b):
                # sel[g] = (maxpool3x3(alpha) > 0.1) * fire, staged to DRAM bf16
                alpha = alpha_d[b]
                am = mask.tile([128, 4, WP], F32, tag="am")
                nc.gpsimd.memset(am[:], 0.0)
                # center rows contiguously, halo rows via partition shifts
                nc.scalar.dma_start(
                    out=am[0:nst, 1:3, 1 : W + 1],
                    in_=alpha.rearrange("(p t) w -> p t w", t=2),
                )
                nc.scalar.dma_start(out=am[1:nst, 0, :], in_=am[0 : nst - 1, 2, :])
                nc.scalar.dma_start(out=am[0 : nst - 1, 3, :], in_=am[1:nst, 1, :])
                mh = mask.tile([128, 4, W], F32, tag="mh")
                nc.vector.tensor_tensor(mh[0:nst], am[0:nst, :, 0:W], am[0:nst, :, 1 : W + 1], ALU.max)
                nc.vector.tensor_tensor(mh[0:nst], mh[0:nst], am[0:nst, :, 2 : W + 2], ALU.max)
                mv = mask.tile([128, 2, W], F32, tag="mv")
                nc.vector.tensor_tensor(mv[0:nst], mh[0:nst, 0:2, :], mh[0:nst, 1:3, :], ALU.max)
                nc.vector.tensor_tensor(mv[0:nst], mv[0:nst], mh[0:nst, 2:4, :], ALU.max)
                fire_t = mask.tile([128, 2, W], F32, tag="fire")
                nc.scalar.dma_start(out=fire_t[0:nst], in_=fire_d[b].rearrange("(p t) w -> p t w", t=2))
                sel_t = mask.tile([128, 2, W], BF16, tag="sel")
                nc.vector.scalar_tensor_tensor(
                    out=sel_t[0:nst], in0=mv[0:nst], scalar=0.1, in1=fire_t[0:nst],
                    op0=ALU.is_gt, op1=ALU.mult,
                )
                sel_dd = dramp.tile([nst, 2 * W], BF16, tag="seld", name=f"seld{b}")
                nc.scalar.dma_start(out=sel_dd[:], in_=sel_t[0:nst].rearrange("p t w -> p (t w)"))
                return sel_dd

            # ---- software-pipelined MLP emission ----
            # pair = 2 groups, 4 image rows apart (rows rr..rr+1 and
            # rr+4..rr+5) loaded by ONE 7-block sliding-window DMA (junk
            # block fills the partition gap 48-63).  Stages are emitted one
            # pair apart so the in-order PE queue never waits on a fresh
            # ACT/DVE product.
            npair_img = nst // 2
            NP = b_loc * npair_img
            st = {}
            sel_dds = {}
            ps3s = {}
            xts = {}

            # PE warmup operand: no-dep matmuls keep the HAM clock-gate
            # open while the pipeline fills (emitted inside early stage1s,
            # overwritten by the real start=True matmuls)
            zt = const.tile([128, 64], BF16)
            nc.vector.memset(zt[:], 0.0)

            def stage1(idx):
                b, pl = divmod(idx, npair_img)
                sub = pl % 2

                def fetch_xt(bb, blkno):
                    xpb = xp_d[bb]
                    ch_stride = (h + 2) * WP
                    xt = xtp.tile([128, 4, WP], BF16, tag="xt", name="xt")
                    src = _mkap(
                        xpb, 8 * blkno * WP,
                        [[WP, 7], [ch_stride, C], [WP, 4], [1, WP]],
                    )
                    eng = nc.sync if blkno % 2 == 0 else nc.gpsimd
                    eng.dma_start(out=xt[0:112, :, :], in_=src)
                    xts[(bb, blkno)] = xt

                if (b, pl // 2) not in xts:
                    fetch_xt(b, pl // 2)
                xt = xts[(b, pl // 2)] if sub == 0 else xts.pop((b, pl // 2))
                if sub == 1 and idx + 2 < NP:
                    # prefetch the tile needed ~2 pairs ahead
                    nb, npl = divmod(idx + 2, npair_img)
                    nblk = npl // 2
                    if (nb, nblk) not in xts:
                        fetch_xt(nb, nblk)
                ps1 = ps1p.tile([128, 2, 512], F32)
                if idx < 8:
                    for _ in range(12):
                        nc.tensor.matmul(
                            out=ps1[0:64, 0, 0:64], lhsT=zt[:], rhs=zt[:],
                            start=True, stop=True,
                        )
                for i in range(3):
                    for half in range(2):
                        pbase = 64 * half
                        nc.tensor.matmul(
                            out=ps1[:, half, :],
                            lhsT=wcat_t[pbase : pbase + 48, i, :],
                            rhs=xt[pbase : pbase + 48, 2 * sub : 2 * sub + 2, i : W + i],
                            start=(i == 0), stop=(i == 2),
                            tile_position=(pbase, 0),
                        )
                h1 = act.tile([EMB, 2, 512], BF16, tag="h1", name="h1")
                nc.scalar.activation(out=h1[:], in_=ps1[:], func=AF.Relu, bias=b1_t[:])
                st[idx] = [h1]

            def stage2(idx):
                (h1,) = st[idx]
                ps2 = ps2p.tile([128, 2, 512], F32)
                for half in range(2):
                    nc.tensor.matmul(
                        out=ps2[:, half, :], lhsT=w2_t[:], rhs=h1[:, half, :],
                        start=True, stop=True,
                    )
                h2 = act.tile([EMB, 2, 512], BF16, tag="h2", name="h2")
                if relu2_act_every and idx % relu2_act_every == 0:
                    nc.scalar.activation(out=h2[:], in_=ps2[:], func=AF.Relu, bias=b2_t[:])
                else:
                    nc.vector.tensor_scalar(h2[:], ps2[:], b2_t[:], 0.0, ALU.add, ALU.max)
                st[idx] = [h2]

            def stage3(idx):
                b, pl = divmod(idx, npair_img)
                (h2,) = st.pop(idx)
                blk = pl // 2
                sub = pl % 2
                if sub == 0:
                    ps3s[(b, blk)] = ps3p.tile([128, 512], F32, name="ps3")
                ps3 = ps3s[(b, blk)]
                for half in range(2):
                    j = 2 * half + sub
                    nc.tensor.matmul(
                        out=ps3[32 * j : 32 * j + 32, :], lhsT=w3_t[:],
                        rhs=h2[:, half, :],
                        start=True, stop=True, tile_position=(0, 32 * j),
                    )
                if sub == 1:
                    ps3s.pop((b, blk))
                    g0 = 4 * blk
                    selb = outp.tile([128, 512], BF16, tag="selb", name="selb")
                    nc.gpsimd.dma_start(
                        out=selb[:],
                        in_=sel_dds[b][g0 : g0 + 4, None, :].to_broadcast([4, 32, 2 * W]),
                    )
                    osb = outp.tile([128, 512], F32, tag="osb", name="osb")
                    nc.vector.scalar_tensor_tensor(
                        out=osb[:], in0=ps3[:], scalar=b3r_t[:], in1=selb[:],
                        op0=ALU.add, op1=ALU.mult,
                    )
                    eng = nc.sync if blk % 2 == 0 else nc.gpsimd
                    eng.dma_start(out=out_d[b, blk], in_=osb[:])

            for b in range(b_loc):
                sel_dds[b] = emit_mask(b)
            for p in range(NP + 2):
                if p < NP:
                    stage1(p)
                if 1 <= p <= NP:
                    stage2(p - 1)
                if 2 <= p <= NP + 1:
                    stage3(p - 2)

    nc.compile()
    return nc


def host_weights(w1, b1, w2, b2, w3, b3):
    sob = np.array([[-1.0, 0, 1], [-2, 0, 2], [-1, 0, 1]], np.float32)
    kx, ky = sob, sob.T
    w1 = np.asarray(w1, np.float32)
    w1a, w1b, w1c = w1[0:C], w1[C : 2 * C], w1[2 * C : 3 * C]
    wcat48 = np.zeros((48, 3, EMB), np.float32)
    for i, dx in enumerate((-1, 0, 1)):
        for blk, dy in enumerate((-1, 0, 1)):
            m = kx[dy + 1, dx + 1] * w1b + ky[dy + 1, dx + 1] * w1c
            if dy == 0 and dx == 0:
                m = m + w1a
            wcat48[16 * blk : 16 * blk + 16, i, :] = m
    wcat = np.zeros((128, 3, EMB), np.float32)
    wcat[0:48] = wcat48
    wcat[64:112] = wcat48
    b3r = np.zeros((128, 1), np.float32)
    for j in range(4):
        b3r[32 * j : 32 * j + 16, 0] = np.asarray(b3, np.float32).reshape(C)
    return {
        "wcat": wcat.astype(ml_dtypes.bfloat16),
        "w2": np.asarray(w2, np.float32).astype(ml_dtypes.bfloat16),
        "w3": np.pad(np.asarray(w3, np.float32), ((0, 0), (0, 16))).astype(
            ml_dtypes.bfloat16
        ),
        "b1": np.asarray(b1, np.float32).reshape(EMB, 1),
        "b2": np.asarray(b2, np.float32).reshape(EMB, 1),
        "b3r": b3r,
    }


def host_x(x, h=H):
    """Pad to [*, C, h+2, W+2] bf16 with a zero border."""
    b = x.shape[0]
    xp = np.zeros((b, C, h + 2, WP), ml_dtypes.bfloat16)
    xp[:, :, 1 : h + 1, 1 : W + 1] = x.astype(ml_dtypes.bfloat16)
    return xp


_nc_cache = {}


def _get_nc():
    if "nc" not in _nc_cache:
        _nc_cache["nc"] = build()
    return _nc_cache["nc"]


def make_in_maps(x, fire_mask, w1, b1, w2, b2, w3, b3):
    x = np.ascontiguousarray(np.asarray(x), np.float32)
    fire = np.ascontiguousarray(np.asarray(fire_mask), np.float32)
    wts = host_weights(w1, b1, w2, b2, w3, b3)
    xp = host_x(x)
    in_maps = []
    for c in range(N_CORES):
        sl = slice(B_LOC * c, B_LOC * (c + 1))
        in_maps.append({
            "xp": xp[sl],
            "alpha": np.ascontiguousarray(x[sl, 3]),
            "fire": np.ascontiguousarray(fire[sl, 0]),
            **wts,
        })
    return in_maps


def unstage(out_stage, h=H):
    """[b, h//8, 128, 512] staging -> [b, C, h, W]."""
    b = out_stage.shape[0]
    v = out_stage.reshape(b, h // 8, 4, 32, 2, W)[:, :, :, 0:C]
    return np.ascontiguousarray(
        v.transpose(0, 3, 1, 2, 4, 5).reshape(b, C, h, W)
    )


def kernel(x, fire_mask, w1, b1, w2, b2, w3, b3):
    nc = _get_nc()
    in_maps = make_in_maps(x, fire_mask, w1, b1, w2, b2, w3, b3)
    res = run_bass_kernel_spmd(nc, in_maps, core_ids=list(range(N_CORES)))
    return np.concatenate(
        [unstage(res.results[c]["out"]) for c in range(N_CORES)], axis=0
    )
